# revision 17
# baseline (speedup 1.0000x reference)
"""Trainium2 Bass kernel for nn_AdaptiveSampler.

Reference computation (per batch element b of BT=64):
  1. seed  = bilinear_sample(features[b], keypoints[b])          # [C, J]
  2. h     = relu(w1 @ seed + b1)                                # [128, J]
     off   = w2 @ h + b2                                        # [2N, J] (pixel units)
  3. samp  = bilinear_sample(features[b], keypoints + off)       # [C, J*N]
  4. out[b] = samp rearranged to [J, N*C]

Strategy: pure data-parallel over BT across 8 NeuronCores (8 batches/core,
processed in 4 pairs).  Per pair, the 8 MiB feature block is DMA'd once into
SBUF in its natural layout [128 part = channel-pair, free = (b_lo, c_lo, y, x)].
All bilinear gathers are done with the gpsimd `ap_gather` extended instruction
(free-dim gather with an int16 index table, uniform across partitions).  The
data-dependent second-pass indices are computed on-chip from the MLP output:
floor/clip/weights on DVE, index table wrapped into the [16, n/16] layout via
one SBUF->SBUF DMA and replicated across partition groups with one PE matmul.
Bilinear weights are folded into a per-sample weight row broadcast across
partitions; the 4-neighbor combine is a DVE multiply + segmented reduce.
Output [J*N, C] is produced by PE transposes of the [C, J*N] result.

Memory roofline: features are read exactly once (256 MiB / 8 cores / ~360GB/s
~= 93 us); everything else overlaps under the feature stream.
"""

import os
import sys

import numpy as np

sys.path.insert(0, "/opt/trn_rl_repo")

BT, C, H, W = 64, 256, 64, 64
J, NPTS = 17, 4
NCORES = 8
BPC = BT // NCORES          # batches per core
NPAIR = BPC // 2            # pairs per core
O2 = 2 * NPTS               # 8 MLP output rows (n, axis)
S1 = 2 * J                  # pass-1 samples per pair (b_lo, pt) = 34
S2 = 2 * J * NPTS           # pass-2 samples per pair (b_lo, pt, n) = 136
I1 = S1 * 2 * 4             # pass-1 gather idx count per pair = 272
I2 = S2 * 2 * 4             # pass-2 gather idx count per pair = 1088
FREE = 2 * 2 * H * W        # feat pair tile free size = 16384
MAGIC = 8388608.0           # 2^23: float32 round-to-nearest-int trick

_CACHE = {}
LAST_RESULTS = None


def _build():
    import concourse.bass as bass
    import concourse.tile as tile
    from concourse import bacc, mybir
    from concourse.ap import AP
    from concourse.masks import make_identity

    dt = mybir.dt
    f32 = dt.float32
    i16 = dt.int16
    Alu = mybir.AluOpType
    Act = mybir.ActivationFunctionType

    nc = bacc.Bacc("TRN2", target_bir_lowering=False, debug=False,
                   num_devices=NCORES)

    feats = nc.dram_tensor("features", [BPC, C, H, W], f32,
                           kind="ExternalInput").ap()
    # pixel-space keypoint bases: [NPAIR, 2*S2] laid out
    # (pair) x (axis, b_lo, pt, n)
    basep = nc.dram_tensor("base_pix", [1, NPAIR * 2 * S2], f32,
                           kind="ExternalInput").ap()
    w1eT = nc.dram_tensor("w1eT", [128, 128], f32, kind="ExternalInput").ap()
    w1oT = nc.dram_tensor("w1oT", [128, 128], f32, kind="ExternalInput").ap()
    w2T = nc.dram_tensor("w2T", [128, O2], f32, kind="ExternalInput").ap()
    b1d = nc.dram_tensor("b1", [128, 1], f32, kind="ExternalInput").ap()
    b2d = nc.dram_tensor("b2", [128, O2], f32, kind="ExternalInput").ap()
    repl = nc.dram_tensor("repl16", [16, 128], f32, kind="ExternalInput").ap()
    out = nc.dram_tensor("out", [BPC, J, NPTS * C], f32,
                         kind="ExternalOutput").ap()

    # DRAM views
    # per (b, c_lo): [128 part, 4096] with partition stride 2 channels
    feat_v = feats.rearrange("bb (p cl) yy xx -> bb p cl (yy xx)", cl=2)
    out_v = out.rearrange("b j (n c) -> b (j n) c", c=C)

    from contextlib import ExitStack

    with tile.TileContext(nc) as tc, ExitStack() as ctx:
        const = ctx.enter_context(tc.tile_pool(name="const", bufs=1))
        featp = ctx.enter_context(tc.tile_pool(name="featp", bufs=2))
        gath = ctx.enter_context(tc.tile_pool(name="gath", bufs=2))
        rowp = ctx.enter_context(tc.tile_pool(name="rowp", bufs=1))
        wbp = ctx.enter_context(tc.tile_pool(name="wbp", bufs=2))
        outp = ctx.enter_context(tc.tile_pool(name="outp", bufs=2))
        psum = ctx.enter_context(tc.tile_pool(name="psum", bufs=2,
                                              space="PSUM"))

        # ---- constants ----
        ident = const.tile([128, 128], f32, tag="ident")
        make_identity(nc, ident[:])
        w1e_t = const.tile([128, 128], f32, tag="w1e")
        nc.sync.dma_start(w1e_t[:], w1eT)
        w1o_t = const.tile([128, 128], f32, tag="w1o")
        nc.sync.dma_start(w1o_t[:], w1oT)
        w2_t = const.tile([128, O2], f32, tag="w2")
        nc.sync.dma_start(w2_t[:], w2T)
        b1_t = const.tile([128, 1], f32, tag="b1")
        nc.sync.dma_start(b1_t[:], b1d)
        b2_t = const.tile([128, O2], f32, tag="b2")
        nc.sync.dma_start(b2_t[:], b2d)
        repl_t = const.tile([16, 128], f32, tag="repl")
        nc.sync.dma_start(repl_t[:], repl)
        base_t = const.tile([1, NPAIR * 2 * S2], f32, tag="base")
        nc.sync.dma_start(base_t[:], basep)

        def chain(pool, x, n, tagpfx):
            """Bilinear helper on [1, n] rows (x may stack axes along free).

            Returns (c0, c1, w0, w1): clipped floor / floor+1 coords and
            validity-folded weights, all [1, n] f32.
            """
            s1 = pool.tile([1, n], f32, tag=tagpfx + "s1")
            s2 = pool.tile([1, n], f32, tag=tagpfx + "s2")
            s3 = pool.tile([1, n], f32, tag=tagpfx + "s3")
            c0 = pool.tile([1, n], f32, tag=tagpfx + "c0")
            c1 = pool.tile([1, n], f32, tag=tagpfx + "c1")
            w0 = pool.tile([1, n], f32, tag=tagpfx + "w0")
            w1t = pool.tile([1, n], f32, tag=tagpfx + "w1")
            hi = float(W - 1)
            nc.vector.tensor_scalar(s1[:], x, MAGIC, None, Alu.add)
            nc.vector.tensor_scalar(s1[:], s1[:], MAGIC, None, Alu.subtract)
            nc.vector.tensor_tensor(s2[:], x, s1[:], Alu.is_lt)
            nc.vector.tensor_sub(s1[:], s1[:], s2[:])              # floor
            nc.vector.tensor_scalar(s3[:], s1[:], 1.0, None, Alu.add)
            nc.vector.tensor_scalar(c0[:], s1[:], 0.0, hi, Alu.max, Alu.min)
            nc.vector.tensor_scalar(c1[:], s3[:], 0.0, hi, Alu.max, Alu.min)
            nc.vector.tensor_tensor(s2[:], s1[:], c0[:], Alu.is_equal)
            nc.vector.tensor_sub(w0[:], s3[:], x)
            nc.vector.tensor_mul(w0[:], w0[:], s2[:])
            nc.vector.tensor_tensor(s2[:], s3[:], c1[:], Alu.is_equal)
            nc.vector.tensor_sub(w1t[:], x, s1[:])
            nc.vector.tensor_mul(w1t[:], w1t[:], s2[:])
            return c0, c1, w0, w1t

        def assemble(pool, nsamp, cx, cy, wx, wy, tagpfx):
            """Build gather-index and weight rows from per-sample coords.

            cx/cy/wx/wy: pairs (lo, hi) of [1, nsamp] AP rows with sample
            order (b_lo, samp-within-b_lo); nb = nsamp/2 samples per b_lo.

            Gather-slot order is i = samp*16 + b_lo*8 + c_lo*4 + quad, so the
            int16 index-storage tile [16, nb] (idx for slot i stored at
            partition i%16, col i//16) is a plain reshape of a row holding
            e-order e = (8*b_lo + 4*c_lo + q)*nb + samp.  The weight row is
            produced directly in slot order i.
            """
            nb = nsamp // 2
            ni = 8 * nsamp           # total gather slots (= 16 * nb)
            idxe = pool.tile([1, ni], f32, tag=tagpfx + "idx")
            wrow = pool.tile([1, ni], f32, tag=tagpfx + "wrow")
            it, wt = idxe[:].tensor, wrow[:].tensor
            io, wo = idxe[:].offset, wrow[:].offset

            def src2(ap, boff):
                return AP(ap.tensor, ap.offset + boff,
                          [list(ap.ap[0]), [0, 2], [1, nb]])

            for b_lo in range(2):
                for q in range(4):
                    qy, qx = q // 2, q % 2
                    ysrc = src2(cy[qy], b_lo * nb)
                    xsrc = src2(cx[qx], b_lo * nb)
                    wysrc = src2(wy[qy], b_lo * nb)
                    wxsrc = src2(wx[qx], b_lo * nb)
                    idst = AP(it, io + nb * (8 * b_lo + q),
                              [[ni, 1], [4 * nb, 2], [1, nb]])
                    wdst = AP(wt, wo + 8 * b_lo + q,
                              [[ni, 1], [4, 2], [16, nb]])
                    nc.vector.scalar_tensor_tensor(idst, ysrc, float(W),
                                                   xsrc, Alu.mult, Alu.add)
                    nc.vector.tensor_mul(wdst, wysrc, wxsrc)
            for b_lo in range(2):
                for c_lo in range(2):
                    sl = idxe[:, 4 * nb * (2 * b_lo + c_lo):
                              4 * nb * (2 * b_lo + c_lo + 1)]
                    nc.vector.tensor_scalar(
                        sl, sl, float(b_lo * 8192 + c_lo * 4096), None,
                        Alu.add)
            return idxe, wrow

        def wrap_idx(pool, idx_row, ni, tagpfx):
            """f32 idx row [1, ni] (e-order) -> int16 idx tile [128, ni/16]."""
            nf = ni // 16
            wrapt = pool.tile([16, nf], f32, tag=tagpfx + "wrap")
            nc.sync.dma_start(wrapt[:], idx_row[:])
            rep_ps = psum.tile([128, nf], f32, tag="replps")
            nc.tensor.matmul(rep_ps[:], repl_t[:], wrapt[:], start=True,
                             stop=True)
            idxt = pool.tile([128, nf], i16, tag=tagpfx + "idxi")
            nc.vector.tensor_copy(idxt[:], rep_ps[:])
            return idxt

        def bcast(pool, row, ni, tagpfx):
            t = pool.tile([128, ni], f32, tag=tagpfx + "wb")
            src = AP(row[:].tensor, row[:].offset,
                     [[ni, 1], [0, 128], [1, ni]])
            nc.sync.dma_start(t[:], src)
            return t

        # ---- pass-1 index/weight prep for all pairs (depends only on coords)
        # base_t rows = pairs; cols = (axis, b_lo, pt, n).  n==0 slice gives
        # keypoint positions.
        base_v = base_t[:].rearrange("a (k x s n) -> a k x s n", k=NPAIR,
                                     x=2, n=4)
        idx1 = []
        w1b = []
        for k in range(NPAIR):
            xk = rowp.tile([1, S1], f32, tag="p1x")
            yk = rowp.tile([1, S1], f32, tag="p1y")
            nc.vector.tensor_copy(xk[:], base_v[:, k, 0, :, 0])
            nc.vector.tensor_copy(yk[:], base_v[:, k, 1, :, 0])
            xc0, xc1, wx0, wx1 = chain(rowp, xk[:], S1, "p1cx")
            yc0, yc1, wy0, wy1 = chain(rowp, yk[:], S1, "p1cy")
            idx_row, w_row = assemble(rowp, S1, (xc0[:], xc1[:]),
                                      (yc0[:], yc1[:]), (wx0[:], wx1[:]),
                                      (wy0[:], wy1[:]), "p1a")
            idx1.append(wrap_idx(const, idx_row, I1, f"p1i{k}"))
            w1b.append(bcast(const, w_row, I1, f"p1w{k}"))

        # ---- main loop over pairs ----
        for k in range(NPAIR):
            feat_t = featp.tile([128, FREE], f32, tag="feat")
            fv = feat_t[:].rearrange("p (bl cl s) -> p bl cl s", bl=2, cl=2)
            for b_lo in range(2):
                for c_lo in range(2):
                    nc.sync.dma_start(fv[:, b_lo, c_lo, :],
                                      feat_v[2 * k + b_lo, :, c_lo, :])

            # pass-1 gather + bilinear combine -> seed [128, (b_lo, c_lo, 17)]
            g1 = gath.tile([128, I1], f32, tag="g1")
            nc.gpsimd.ap_gather(g1[:], feat_t[:], idx1[k][:], channels=128,
                                num_elems=FREE, d=1, num_idxs=I1)
            nc.vector.tensor_mul(g1[:], g1[:], w1b[k][:])
            seed = gath.tile([128, I1 // 4], f32, tag="seed")
            nc.vector.reduce_sum(
                seed[:], g1[:].rearrange("p (g q) -> p g q", q=4),
                axis=mybir.AxisListType.X)
            # seed cols are (pt, b_lo, c_lo): col = pt*4 + b_lo*2 + c_lo
            seed_v = seed[:].rearrange("p (g r) -> p g r", r=4)

            # MLP: h = relu(w1 @ seed + b1) ; off = w2 @ h + b2
            h_ps = psum.tile([128, S1], f32, tag="hps")
            for b_lo in range(2):
                hsl = h_ps[:, b_lo * J:(b_lo + 1) * J]
                ssl = seed_v[:, :, 2 * b_lo]
                osl = seed_v[:, :, 2 * b_lo + 1]
                nc.tensor.matmul(hsl, w1e_t[:], ssl, start=True, stop=False)
                nc.tensor.matmul(hsl, w1o_t[:], osl, start=False, stop=True)
            h_t = gath.tile([128, S1], f32, tag="h")
            nc.scalar.activation(h_t[:], h_ps[:], Act.Relu, bias=b1_t[:, 0:1])
            # transposed layer 2: off^T [34, 8] with partitions (b_lo, pt),
            # free o2 = (n, axis)
            off_ps = psum.tile([S1, O2], f32, tag="offps")
            nc.tensor.matmul(off_ps[:], h_t[:], w2_t[:], start=True, stop=True)
            off_t = gath.tile([S1, O2], f32, tag="off")
            nc.vector.tensor_add(off_t[:], off_ps[:], b2_t[0:S1, :])

            # flatten off -> xy row [1, 2*S2]: x cols 0:S2, y cols S2:2*S2,
            # sample order (b_lo, pt, n)
            offrow = rowp.tile([1, 2 * S2], f32, tag="p2off")
            for axis in range(2):
                src = AP(off_t[:].tensor, off_t[:].offset + axis,
                         [[O2, S1], [2, NPTS]])
                nc.sync.dma_start(offrow[:, axis * S2:(axis + 1) * S2], src)
            xy = rowp.tile([1, 2 * S2], f32, tag="p2xy")
            nc.vector.tensor_add(xy[:], offrow[:],
                                 base_t[:, k * 2 * S2:(k + 1) * 2 * S2])

            c0, c1, w0, w1_ = chain(rowp, xy[:], 2 * S2, "p2c")
            # x parts: cols 0:S2 of c*/w*; y parts: cols S2:2*S2
            idx_row, w_row = assemble(
                rowp, S2,
                (c0[:, 0:S2], c1[:, 0:S2]),
                (c0[:, S2:2 * S2], c1[:, S2:2 * S2]),
                (w0[:, 0:S2], w1_[:, 0:S2]),
                (w0[:, S2:2 * S2], w1_[:, S2:2 * S2]), "p2a")
            idx2 = wrap_idx(wbp, idx_row, I2, "p2i")
            w2b = bcast(wbp, w_row, I2, "p2w")

            # pass-2 gather + combine -> samp [128, (b_lo, c_lo, pt, n)]
            g2 = gath.tile([128, I2], f32, tag="g2")
            nc.gpsimd.ap_gather(g2[:], feat_t[:], idx2[:], channels=128,
                                num_elems=FREE, d=1, num_idxs=I2)
            nc.vector.tensor_mul(g2[:], g2[:], w2b[:])
            samp = gath.tile([128, I2 // 4], f32, tag="samp")
            nc.vector.reduce_sum(
                samp[:], g2[:].rearrange("p (g q) -> p g q", q=4),
                axis=mybir.AxisListType.X)
            # samp cols are (ptn, b_lo, c_lo): col = ptn*4 + b_lo*2 + c_lo
            samp_v = samp[:].rearrange("p (g r) -> p g r", r=4)

            # output: per (b_lo, c_lo) transpose [128, 68] -> [68, 128],
            # interleave parity into [68, 256], DMA out.
            for b_lo in range(2):
                ot = outp.tile([J * NPTS, C], f32, tag="ot")
                ov = ot[:].rearrange("q (c cl) -> q c cl", cl=2)
                for c_lo in range(2):
                    t_ps = psum.tile([J * NPTS, 128], f32, tag="tps")
                    nc.tensor.transpose(t_ps[:], samp_v[:, :, 2 * b_lo + c_lo],
                                        ident[:])
                    nc.vector.tensor_copy(ov[:, :, c_lo], t_ps[:])
                nc.sync.dma_start(out_v[2 * k + b_lo], ot[:])

    nc.compile()
    return nc


def _host_prep(features, keypoint_coords, w1, b1, w2, b2):
    """Shard + lay out inputs for the 8 cores."""
    f32 = np.float32
    feats = np.ascontiguousarray(features.reshape(NCORES, BPC, C, H, W),
                                 dtype=f32)
    pix = (np.asarray(keypoint_coords, f32) + 1.0) * 0.5 * (W - 1)  # [BT,J,2]
    # base_pix per core: [NPAIR, (axis, b_lo, pt, n)]
    bp = pix.reshape(NCORES, NPAIR, 2, J, 2)            # [core, k, b_lo, pt, ax]
    bp = bp.transpose(0, 1, 4, 2, 3)                     # [core, k, ax, b_lo, pt]
    bp = np.repeat(bp[..., None], NPTS, axis=-1)         # [..., n]
    bp = np.ascontiguousarray(bp.reshape(NCORES, 1, NPAIR * 2 * S2), f32)

    w1T = np.asarray(w1, f32).T                          # [256, 128]
    w1eT = np.ascontiguousarray(w1T[0::2])               # even channels
    w1oT = np.ascontiguousarray(w1T[1::2])
    w2T = np.ascontiguousarray(np.asarray(w2, f32).T)    # [128, 8]
    b1c = np.ascontiguousarray(np.asarray(b1, f32)[:, None])
    b2c = np.ascontiguousarray(
        np.tile(np.asarray(b2, f32)[None, :], (128, 1)))
    repl16 = (np.arange(128)[None, :] % 16 ==
              np.arange(16)[:, None]).astype(f32)

    in_maps = []
    for i in range(NCORES):
        in_maps.append({
            "features": feats[i],
            "base_pix": bp[i],
            "w1eT": w1eT,
            "w1oT": w1oT,
            "w2T": w2T,
            "b1": b1c,
            "b2": b2c,
            "repl16": repl16,
        })
    return in_maps


def kernel(features, keypoint_coords, w1, b1, w2, b2):
    global LAST_RESULTS
    from concourse.bass_utils import run_bass_kernel_spmd

    if "nc" not in _CACHE:
        _CACHE["nc"] = _build()
    nc = _CACHE["nc"]
    in_maps = _host_prep(features, keypoint_coords, w1, b1, w2, b2)
    res = run_bass_kernel_spmd(nc, in_maps, core_ids=list(range(NCORES)))
    LAST_RESULTS = res
    out = np.concatenate([res.results[i]["out"] for i in range(NCORES)],
                         axis=0)
    return out.astype(np.float32)


if __name__ == "__main__":
    nc = _build()
    print("build + compile OK")


# revision 19
# speedup vs baseline: 1.0395x; 1.0395x over previous
"""Trainium2 Bass kernel for nn_AdaptiveSampler.

Reference computation (per batch element b of BT=64):
  1. seed  = bilinear_sample(features[b], keypoints[b])          # [C, J]
  2. h     = relu(w1 @ seed + b1)                                # [128, J]
     off   = w2 @ h + b2                                        # [2N, J] (pixel units)
  3. samp  = bilinear_sample(features[b], keypoints + off)       # [C, J*N]
  4. out[b] = samp rearranged to [J, N*C]

Strategy: pure data-parallel over BT across 8 NeuronCores (8 batches/core,
processed in 4 pairs).  Per pair, the 8 MiB feature block is DMA'd once into
SBUF in its natural layout [128 part = channel-pair, free = (b_lo, c_lo, y, x)].
All bilinear gathers are done with the gpsimd `ap_gather` extended instruction
(free-dim gather with an int16 index table, uniform across partitions).  The
data-dependent second-pass indices are computed on-chip from the MLP output:
floor/clip/weights on DVE, index table wrapped into the [16, n/16] layout via
one SBUF->SBUF DMA and replicated across partition groups with one PE matmul.
Bilinear weights are folded into a per-sample weight row broadcast across
partitions; the 4-neighbor combine is a DVE multiply + segmented reduce.
Output [J*N, C] is produced by PE transposes of the [C, J*N] result.

Memory roofline: features are read exactly once (256 MiB / 8 cores / ~360GB/s
~= 93 us); everything else overlaps under the feature stream.
"""

import os
import sys

import numpy as np

sys.path.insert(0, "/opt/trn_rl_repo")

BT, C, H, W = 64, 256, 64, 64
J, NPTS = 17, 4
NCORES = 8
BPC = BT // NCORES          # batches per core
NPAIR = BPC // 2            # pairs per core
O2 = 2 * NPTS               # 8 MLP output rows (n, axis)
S1 = 2 * J                  # pass-1 samples per pair (b_lo, pt) = 34
S2 = 2 * J * NPTS           # pass-2 samples per pair (b_lo, pt, n) = 136
I1 = S1 * 2 * 4             # pass-1 gather idx count per pair = 272
I2 = S2 * 2 * 4             # pass-2 gather idx count per pair = 1088
FREE = 2 * 2 * H * W        # feat pair tile free size = 16384
MAGIC = 8388608.0           # 2^23: float32 round-to-nearest-int trick

_CACHE = {}
LAST_RESULTS = None


def _build():
    import concourse.bass as bass
    import concourse.tile as tile
    from concourse import bacc, mybir
    from concourse.ap import AP
    from concourse.masks import make_identity

    dt = mybir.dt
    f32 = dt.float32
    i16 = dt.int16
    Alu = mybir.AluOpType
    Act = mybir.ActivationFunctionType

    nc = bacc.Bacc("TRN2", target_bir_lowering=False, debug=False,
                   num_devices=NCORES)

    feats = nc.dram_tensor("features", [BPC, C, H, W], f32,
                           kind="ExternalInput").ap()
    # pixel-space keypoint bases: [NPAIR, 2*S2] laid out
    # (pair) x (axis, b_lo, pt, n)
    basep = nc.dram_tensor("base_pix", [1, NPAIR * 2 * S2], f32,
                           kind="ExternalInput").ap()
    w1eT = nc.dram_tensor("w1eT", [128, 128], f32, kind="ExternalInput").ap()
    w1oT = nc.dram_tensor("w1oT", [128, 128], f32, kind="ExternalInput").ap()
    w2T = nc.dram_tensor("w2T", [128, O2], f32, kind="ExternalInput").ap()
    b1d = nc.dram_tensor("b1", [128, 1], f32, kind="ExternalInput").ap()
    b2d = nc.dram_tensor("b2", [128, O2], f32, kind="ExternalInput").ap()
    repl = nc.dram_tensor("repl16", [16, 128], f32, kind="ExternalInput").ap()
    out = nc.dram_tensor("out", [BPC, J, NPTS * C], f32,
                         kind="ExternalOutput").ap()

    # DRAM views
    # per b: [128 part, 8192] free (c_lo, y, x); 32 KiB contiguous runs
    feat_v = feats.rearrange("b (p q) yy xx -> b p (q yy xx)", q=2)
    out_v = out.rearrange("b j (n c) -> b (j n) c", c=C)

    from contextlib import ExitStack

    with tile.TileContext(nc) as tc, ExitStack() as ctx:
        const = ctx.enter_context(tc.tile_pool(name="const", bufs=1))
        featp = ctx.enter_context(tc.tile_pool(name="featp", bufs=2))
        gath = ctx.enter_context(tc.tile_pool(name="gath", bufs=2))
        rowp = ctx.enter_context(tc.tile_pool(name="rowp", bufs=1))
        wbp = ctx.enter_context(tc.tile_pool(name="wbp", bufs=2))
        outp = ctx.enter_context(tc.tile_pool(name="outp", bufs=2))
        psum = ctx.enter_context(tc.tile_pool(name="psum", bufs=2,
                                              space="PSUM"))
        psum1 = ctx.enter_context(tc.tile_pool(name="psum1", bufs=1,
                                               space="PSUM"))

        # ---- constants ----
        ident = const.tile([128, 128], f32, tag="ident")
        make_identity(nc, ident[:])
        w1e_t = const.tile([128, 128], f32, tag="w1e")
        nc.sync.dma_start(w1e_t[:], w1eT)
        w1o_t = const.tile([128, 128], f32, tag="w1o")
        nc.sync.dma_start(w1o_t[:], w1oT)
        w2_t = const.tile([128, O2], f32, tag="w2")
        nc.sync.dma_start(w2_t[:], w2T)
        b1_t = const.tile([128, 1], f32, tag="b1")
        nc.sync.dma_start(b1_t[:], b1d)
        b2_t = const.tile([128, O2], f32, tag="b2")
        nc.sync.dma_start(b2_t[:], b2d)
        repl_t = const.tile([16, 128], f32, tag="repl")
        nc.sync.dma_start(repl_t[:], repl)
        base_t = const.tile([1, NPAIR * 2 * S2], f32, tag="base")
        nc.sync.dma_start(base_t[:], basep)
        ones_t = const.tile([1, 128], f32, tag="ones")
        nc.vector.memset(ones_t[:], 1.0)

        def repl_w(row, ni, tag):
            """Replicate [1, ni] row to all partitions via PE -> PSUM."""
            wps = psum1.tile([128, ni], f32, tag=tag)
            for c0 in range(0, ni, 512):
                c1 = min(c0 + 512, ni)
                nc.tensor.matmul(wps[:, c0:c1], ones_t[:], row[:, c0:c1],
                                 start=True, stop=True)
            return wps

        def chain(pool, x, n, tagpfx):
            """Bilinear helper on [1, n] rows (x may stack axes along free).

            Returns (c0, c1, w0, w1): clipped floor / floor+1 coords and
            validity-folded weights, all [1, n] f32.
            """
            s1 = pool.tile([1, n], f32, tag=tagpfx + "s1")
            s2 = pool.tile([1, n], f32, tag=tagpfx + "s2")
            s3 = pool.tile([1, n], f32, tag=tagpfx + "s3")
            c0 = pool.tile([1, n], f32, tag=tagpfx + "c0")
            c1 = pool.tile([1, n], f32, tag=tagpfx + "c1")
            w0 = pool.tile([1, n], f32, tag=tagpfx + "w0")
            w1t = pool.tile([1, n], f32, tag=tagpfx + "w1")
            hi = float(W - 1)
            nc.vector.tensor_scalar(s1[:], x, MAGIC, None, Alu.add)
            nc.vector.tensor_scalar(s1[:], s1[:], MAGIC, None, Alu.subtract)
            nc.vector.tensor_tensor(s2[:], x, s1[:], Alu.is_lt)
            nc.vector.tensor_sub(s1[:], s1[:], s2[:])              # floor
            nc.vector.tensor_scalar(s3[:], s1[:], 1.0, None, Alu.add)
            nc.vector.tensor_scalar(c0[:], s1[:], 0.0, hi, Alu.max, Alu.min)
            nc.vector.tensor_scalar(c1[:], s3[:], 0.0, hi, Alu.max, Alu.min)
            nc.vector.tensor_tensor(s2[:], s1[:], c0[:], Alu.is_equal)
            nc.vector.tensor_sub(w0[:], s3[:], x)
            nc.vector.tensor_mul(w0[:], w0[:], s2[:])
            nc.vector.tensor_tensor(s2[:], s3[:], c1[:], Alu.is_equal)
            nc.vector.tensor_sub(w1t[:], x, s1[:])
            nc.vector.tensor_mul(w1t[:], w1t[:], s2[:])
            return c0, c1, w0, w1t

        def assemble(pool, nsamp, cx, cy, wx, wy, tagpfx):
            """Build gather-index and weight rows from per-sample coords.

            cx/cy/wx/wy: pairs (lo, hi) of [1, nsamp] AP rows with sample
            order (b_lo, samp-within-b_lo); nb = nsamp/2 samples per b_lo.

            Gather-slot order is i = samp*16 + b_lo*8 + c_lo*4 + quad, so the
            int16 index-storage tile [16, nb] (idx for slot i stored at
            partition i%16, col i//16) is a plain reshape of a row holding
            e-order e = (8*b_lo + 4*c_lo + q)*nb + samp.  The weight row is
            produced directly in slot order i.
            """
            nb = nsamp // 2
            ni = 8 * nsamp           # total gather slots (= 16 * nb)
            idxe = pool.tile([1, ni], f32, tag=tagpfx + "idx")
            wrow = pool.tile([1, ni], f32, tag=tagpfx + "wrow")
            it, wt = idxe[:].tensor, wrow[:].tensor
            io, wo = idxe[:].offset, wrow[:].offset

            def src2(ap, boff):
                return AP(ap.tensor, ap.offset + boff,
                          [list(ap.ap[0]), [0, 2], [1, nb]])

            for b_lo in range(2):
                for q in range(4):
                    qy, qx = q // 2, q % 2
                    ysrc = src2(cy[qy], b_lo * nb)
                    xsrc = src2(cx[qx], b_lo * nb)
                    wysrc = src2(wy[qy], b_lo * nb)
                    wxsrc = src2(wx[qx], b_lo * nb)
                    idst = AP(it, io + nb * (8 * b_lo + q),
                              [[ni, 1], [4 * nb, 2], [1, nb]])
                    wdst = AP(wt, wo + 8 * b_lo + q,
                              [[ni, 1], [4, 2], [16, nb]])
                    nc.vector.scalar_tensor_tensor(idst, ysrc, float(W),
                                                   xsrc, Alu.mult, Alu.add)
                    nc.vector.tensor_mul(wdst, wysrc, wxsrc)
            for b_lo in range(2):
                for c_lo in range(2):
                    sl = idxe[:, 4 * nb * (2 * b_lo + c_lo):
                              4 * nb * (2 * b_lo + c_lo + 1)]
                    nc.vector.tensor_scalar(
                        sl, sl, float(b_lo * 8192 + c_lo * 4096), None,
                        Alu.add)
            return idxe, wrow

        def wrap_idx(pool, idx_row, ni, tagpfx):
            """f32 idx row [1, ni] (e-order) -> int16 idx tile [128, ni/16]."""
            nf = ni // 16
            wrapt = pool.tile([16, nf], f32, tag=tagpfx + "wrap")
            nc.sync.dma_start(wrapt[:], idx_row[:])
            rep_ps = psum1.tile([128, nf], f32, tag="replps")
            nc.tensor.matmul(rep_ps[:], repl_t[:], wrapt[:], start=True,
                             stop=True)
            idxt = pool.tile([128, nf], i16, tag=tagpfx + "idxi")
            nc.vector.tensor_copy(idxt[:], rep_ps[:])
            return idxt

        # ---- pass-1 index/weight prep for all pairs (depends only on coords)
        # base_t rows = pairs; cols = (axis, b_lo, pt, n).  n==0 slice gives
        # keypoint positions.
        base_v = base_t[:].rearrange("a (k x s n) -> a k x s n", k=NPAIR,
                                     x=2, n=4)
        idx1 = []
        w1rows = []
        for k in range(NPAIR):
            xk = const.tile([1, S1], f32, tag=f"p1x{k}")
            yk = const.tile([1, S1], f32, tag=f"p1y{k}")
            nc.vector.tensor_copy(xk[:], base_v[:, k, 0, :, 0])
            nc.vector.tensor_copy(yk[:], base_v[:, k, 1, :, 0])
            xc0, xc1, wx0, wx1 = chain(rowp, xk[:], S1, "p1cx")
            yc0, yc1, wy0, wy1 = chain(rowp, yk[:], S1, "p1cy")
            idx_row, w_row = assemble(const, S1, (xc0[:], xc1[:]),
                                      (yc0[:], yc1[:]), (wx0[:], wx1[:]),
                                      (wy0[:], wy1[:]), f"p1a{k}")
            idx1.append(wrap_idx(const, idx_row, I1, f"p1i{k}"))
            w1rows.append(w_row)

        # ---- main loop over pairs ----
        for k in range(NPAIR):
            feat_t = featp.tile([128, FREE], f32, tag="feat")
            fv = feat_t[:].rearrange("p (bl s) -> p bl s", bl=2)
            for b_lo in range(2):
                nc.sync.dma_start(fv[:, b_lo, :], feat_v[2 * k + b_lo])

            # pass-1 gather + bilinear combine -> seed [128, (b_lo, c_lo, 17)]
            g1 = gath.tile([128, I1], f32, tag="g1")
            nc.gpsimd.ap_gather(g1[:], feat_t[:], idx1[k][:], channels=128,
                                num_elems=FREE, d=1, num_idxs=I1)
            w1ps = repl_w(w1rows[k][:], I1, "wps")
            nc.vector.tensor_mul(g1[:], g1[:], w1ps[:])
            seed = gath.tile([128, I1 // 4], f32, tag="seed")
            nc.vector.reduce_sum(
                seed[:], g1[:].rearrange("p (g q) -> p g q", q=4),
                axis=mybir.AxisListType.X)
            # seed cols are (pt, b_lo, c_lo): col = pt*4 + b_lo*2 + c_lo
            seed_v = seed[:].rearrange("p (g r) -> p g r", r=4)

            # MLP: h = relu(w1 @ seed + b1) ; off = w2 @ h + b2
            h_ps = psum.tile([128, S1], f32, tag="hps")
            for b_lo in range(2):
                hsl = h_ps[:, b_lo * J:(b_lo + 1) * J]
                ssl = seed_v[:, :, 2 * b_lo]
                osl = seed_v[:, :, 2 * b_lo + 1]
                nc.tensor.matmul(hsl, w1e_t[:], ssl, start=True, stop=False)
                nc.tensor.matmul(hsl, w1o_t[:], osl, start=False, stop=True)
            h_t = gath.tile([128, S1], f32, tag="h")
            nc.scalar.activation(h_t[:], h_ps[:], Act.Relu, bias=b1_t[:, 0:1])
            # transposed layer 2: off^T [34, 8] with partitions (b_lo, pt),
            # free o2 = (n, axis)
            off_ps = psum1.tile([S1, O2], f32, tag="offps")
            nc.tensor.matmul(off_ps[:], h_t[:], w2_t[:], start=True, stop=True)
            off_t = gath.tile([S1, O2], f32, tag="off")
            nc.vector.tensor_add(off_t[:], off_ps[:], b2_t[0:S1, :])

            # flatten off -> xy row [1, 2*S2]: x cols 0:S2, y cols S2:2*S2,
            # sample order (b_lo, pt, n)
            offrow = rowp.tile([1, 2 * S2], f32, tag="p2off")
            for axis in range(2):
                src = AP(off_t[:].tensor, off_t[:].offset + axis,
                         [[O2, S1], [2, NPTS]])
                nc.sync.dma_start(offrow[:, axis * S2:(axis + 1) * S2], src)
            xy = rowp.tile([1, 2 * S2], f32, tag="p2xy")
            nc.vector.tensor_add(xy[:], offrow[:],
                                 base_t[:, k * 2 * S2:(k + 1) * 2 * S2])

            c0, c1, w0, w1_ = chain(rowp, xy[:], 2 * S2, "p2c")
            # x parts: cols 0:S2 of c*/w*; y parts: cols S2:2*S2
            idx_row, w_row = assemble(
                rowp, S2,
                (c0[:, 0:S2], c1[:, 0:S2]),
                (c0[:, S2:2 * S2], c1[:, S2:2 * S2]),
                (w0[:, 0:S2], w1_[:, 0:S2]),
                (w0[:, S2:2 * S2], w1_[:, S2:2 * S2]), "p2a")
            idx2 = wrap_idx(wbp, idx_row, I2, "p2i")
            w2ps = repl_w(w_row[:], I2, "wps")

            # pass-2 gather + combine -> samp [128, (b_lo, c_lo, pt, n)]
            g2 = gath.tile([128, I2], f32, tag="g2")
            nc.gpsimd.ap_gather(g2[:], feat_t[:], idx2[:], channels=128,
                                num_elems=FREE, d=1, num_idxs=I2)
            nc.vector.tensor_mul(g2[:], g2[:], w2ps[:])
            samp = gath.tile([128, I2 // 4], f32, tag="samp")
            nc.vector.reduce_sum(
                samp[:], g2[:].rearrange("p (g q) -> p g q", q=4),
                axis=mybir.AxisListType.X)
            # samp cols are (ptn, b_lo, c_lo): col = ptn*4 + b_lo*2 + c_lo
            samp_v = samp[:].rearrange("p (g r) -> p g r", r=4)

            # output: per (b_lo, c_lo) transpose [128, 68] -> [68, 128],
            # interleave parity into [68, 256], DMA out.
            for b_lo in range(2):
                ot = outp.tile([J * NPTS, C], f32, tag="ot")
                ov = ot[:].rearrange("q (c cl) -> q c cl", cl=2)
                for c_lo in range(2):
                    t_ps = psum1.tile([J * NPTS, 128], f32, tag="tps")
                    nc.tensor.transpose(t_ps[:], samp_v[:, :, 2 * b_lo + c_lo],
                                        ident[:])
                    nc.vector.tensor_copy(ov[:, :, c_lo], t_ps[:])
                nc.sync.dma_start(out_v[2 * k + b_lo], ot[:])

    nc.compile()
    return nc


def _host_prep(features, keypoint_coords, w1, b1, w2, b2):
    """Shard + lay out inputs for the 8 cores."""
    f32 = np.float32
    feats = np.ascontiguousarray(features.reshape(NCORES, BPC, C, H, W),
                                 dtype=f32)
    pix = (np.asarray(keypoint_coords, f32) + 1.0) * 0.5 * (W - 1)  # [BT,J,2]
    # base_pix per core: [NPAIR, (axis, b_lo, pt, n)]
    bp = pix.reshape(NCORES, NPAIR, 2, J, 2)            # [core, k, b_lo, pt, ax]
    bp = bp.transpose(0, 1, 4, 2, 3)                     # [core, k, ax, b_lo, pt]
    bp = np.repeat(bp[..., None], NPTS, axis=-1)         # [..., n]
    bp = np.ascontiguousarray(bp.reshape(NCORES, 1, NPAIR * 2 * S2), f32)

    w1T = np.asarray(w1, f32).T                          # [256, 128]
    w1eT = np.ascontiguousarray(w1T[0::2])               # even channels
    w1oT = np.ascontiguousarray(w1T[1::2])
    w2T = np.ascontiguousarray(np.asarray(w2, f32).T)    # [128, 8]
    b1c = np.ascontiguousarray(np.asarray(b1, f32)[:, None])
    b2c = np.ascontiguousarray(
        np.tile(np.asarray(b2, f32)[None, :], (128, 1)))
    repl16 = (np.arange(128)[None, :] % 16 ==
              np.arange(16)[:, None]).astype(f32)

    in_maps = []
    for i in range(NCORES):
        in_maps.append({
            "features": feats[i],
            "base_pix": bp[i],
            "w1eT": w1eT,
            "w1oT": w1oT,
            "w2T": w2T,
            "b1": b1c,
            "b2": b2c,
            "repl16": repl16,
        })
    return in_maps


def kernel(features, keypoint_coords, w1, b1, w2, b2):
    global LAST_RESULTS
    from concourse.bass_utils import run_bass_kernel_spmd

    if "nc" not in _CACHE:
        _CACHE["nc"] = _build()
    nc = _CACHE["nc"]
    in_maps = _host_prep(features, keypoint_coords, w1, b1, w2, b2)
    res = run_bass_kernel_spmd(nc, in_maps, core_ids=list(range(NCORES)))
    LAST_RESULTS = res
    out = np.concatenate([res.results[i]["out"] for i in range(NCORES)],
                         axis=0)
    return out.astype(np.float32)


if __name__ == "__main__":
    nc = _build()
    print("build + compile OK")


# revision 25
# speedup vs baseline: 1.1131x; 1.0709x over previous
"""Trainium2 Bass kernel for nn_AdaptiveSampler.

Per batch element b of BT=64:
  1. seed  = bilinear_sample(features[b], keypoints[b])          # [C, J]
  2. h     = relu(w1 @ seed + b1); off = w2 @ h + b2             # [2N, J] px
  3. samp  = bilinear_sample(features[b], keypoints + off)       # [C, J*N]
  4. out[b] = samp rearranged to [J, N*C]

Data-parallel over BT across 8 NeuronCores (8 batches/core, 4 pairs).
Features are host-transposed to channel-quad-last layout: SBUF pair tile
[128 part = (b_lo, c//4), free = (y, x, c%4)], loaded once (memory roofline
~90us/core).  Bilinear gathers use the gpsimd `ap_gather` extended
instruction with d=4 (one int16 index per (sample, neighbor) fetches 4
channels), with per-16-partition index groups carrying each b_lo's indices.
Second-pass indices/weights are computed on-chip from the MLP output
(floor/clip via the 2^23 trick on DVE), wrapped into the [32, n/16] storage
layout by one SBUF DMA and replicated across partitions with one PE matmul.
The 4-neighbor combine is one DVE multiply (step-0 channel broadcast of the
weight tile in PSUM) + one strided reduce.  Output [J*N, C] comes from PE
transposes.
"""

import os
import sys

import numpy as np

sys.path.insert(0, "/opt/trn_rl_repo")

BT, C, H, W = 64, 256, 64, 64
J, NPTS = 17, 4
NCORES = 8
BPC = BT // NCORES          # 8 batches per core
NPAIR = BPC // 2
O2 = 2 * NPTS
PT1 = 20                    # padded pass-1 points per b (17 -> 20)
N1 = PT1 * 4                # pass-1 gather slots per group = 80 (F=5)
PTN = J * NPTS              # pass-2 samples per b = 68
N2 = PTN * 4                # pass-2 gather slots per group = 272 (F=17)
FREE = 4096                 # d=4 units per partition (y, x)
MAGIC = 8388608.0

_CACHE = {}
LAST_RESULTS = None


def _build():
    STAGE = os.environ.get("KSTAGE", "full")
    import concourse.bass as bass
    import concourse.tile as tile
    from concourse import bacc, mybir
    from concourse.ap import AP

    dt = mybir.dt
    f32 = dt.float32
    i16 = dt.int16
    Alu = mybir.AluOpType
    Act = mybir.ActivationFunctionType

    nc = bacc.Bacc("TRN2", target_bir_lowering=False, debug=False,
                   num_devices=NCORES)

    feats = nc.dram_tensor("features", [BPC, 64, 4 * H * W], f32,
                           kind="ExternalInput").ap()
    basep = nc.dram_tensor("base_pix", [1, NPAIR * 2 * 2 * PTN], f32,
                           kind="ExternalInput").ap()
    w1qd = nc.dram_tensor("w1q", [128, 512], f32, kind="ExternalInput").ap()
    w2Td = nc.dram_tensor("w2T", [128, O2], f32, kind="ExternalInput").ap()
    b1d = nc.dram_tensor("b1", [128, 1], f32, kind="ExternalInput").ap()
    b2d = nc.dram_tensor("b2", [128, O2], f32, kind="ExternalInput").ap()
    replbd = nc.dram_tensor("replb", [32, 128], f32,
                            kind="ExternalInput").ap()
    onesbd = nc.dram_tensor("onesb", [2, 128], f32, kind="ExternalInput").ap()
    ident2d = nc.dram_tensor("ident2", [128, 64], f32,
                             kind="ExternalInput").ap()
    out = nc.dram_tensor("out", [BPC, J, NPTS * C], f32,
                         kind="ExternalOutput").ap()

    out_v = out.rearrange("b j (n c) -> b (j n) c", c=C)

    from contextlib import ExitStack

    with tile.TileContext(nc) as tc, ExitStack() as ctx:
        const = ctx.enter_context(tc.tile_pool(name="const", bufs=1))
        featp = ctx.enter_context(tc.tile_pool(name="featp", bufs=2))
        gath = ctx.enter_context(tc.tile_pool(name="gath", bufs=2))
        rowp = ctx.enter_context(tc.tile_pool(name="rowp", bufs=1))
        wbp = ctx.enter_context(tc.tile_pool(name="wbp", bufs=2))
        outp = ctx.enter_context(tc.tile_pool(name="outp", bufs=2))
        psum = ctx.enter_context(tc.tile_pool(name="psum", bufs=2,
                                              space="PSUM"))
        psum1 = ctx.enter_context(tc.tile_pool(name="psum1", bufs=1,
                                               space="PSUM"))

        # ---- constants ----
        w1q_t = const.tile([128, 512], f32, tag="w1q")
        nc.sync.dma_start(w1q_t[:], w1qd)
        w2_t = const.tile([128, O2], f32, tag="w2")
        nc.sync.dma_start(w2_t[:], w2Td)
        b1_t = const.tile([128, 1], f32, tag="b1")
        nc.sync.dma_start(b1_t[:], b1d)
        b2_t = const.tile([128, O2], f32, tag="b2")
        nc.sync.dma_start(b2_t[:], b2d)
        replb_t = const.tile([32, 128], f32, tag="replb")
        nc.sync.dma_start(replb_t[:], replbd)
        onesb_t = const.tile([2, 128], f32, tag="onesb")
        nc.sync.dma_start(onesb_t[:], onesbd)
        ident2_t = const.tile([128, 64], f32, tag="ident2")
        nc.sync.dma_start(ident2_t[:], ident2d)
        base_t = const.tile([1, NPAIR * 2 * 2 * PTN], f32, tag="base")
        nc.sync.dma_start(base_t[:], basep)

        def chain(pool, x, n, tagpfx):
            """floor/clip/validity-weights on a [1, n] row; returns
            (c0, c1, w0, w1) tiles [1, n]."""
            s1 = pool.tile([1, n], f32, tag=tagpfx + "s1")
            s2 = pool.tile([1, n], f32, tag=tagpfx + "s2")
            s3 = pool.tile([1, n], f32, tag=tagpfx + "s3")
            c0 = pool.tile([1, n], f32, tag=tagpfx + "c0")
            c1 = pool.tile([1, n], f32, tag=tagpfx + "c1")
            w0 = pool.tile([1, n], f32, tag=tagpfx + "w0")
            w1t = pool.tile([1, n], f32, tag=tagpfx + "w1")
            hi = float(W - 1)
            nc.vector.tensor_scalar(s1[:], x, MAGIC, None, Alu.add)
            nc.vector.tensor_scalar(s1[:], s1[:], MAGIC, None, Alu.subtract)
            nc.vector.tensor_tensor(s2[:], x, s1[:], Alu.is_lt)
            nc.vector.tensor_sub(s1[:], s1[:], s2[:])              # floor
            nc.vector.tensor_scalar(s3[:], s1[:], 1.0, None, Alu.add)
            nc.vector.tensor_scalar(c0[:], s1[:], 0.0, hi, Alu.max, Alu.min)
            nc.vector.tensor_scalar(c1[:], s3[:], 0.0, hi, Alu.max, Alu.min)
            nc.vector.tensor_tensor(s2[:], s1[:], c0[:], Alu.is_equal)
            nc.vector.tensor_sub(w0[:], s3[:], x)
            nc.vector.tensor_mul(w0[:], w0[:], s2[:])
            nc.vector.tensor_tensor(s2[:], s3[:], c1[:], Alu.is_equal)
            nc.vector.tensor_sub(w1t[:], x, s1[:])
            nc.vector.tensor_mul(w1t[:], w1t[:], s2[:])
            return c0, c1, w0, w1t

        def assemble(pool, npt, cx, cy, wx, wy, xoff, tagpfx):
            """Build e-order idx row [1, 2*npt*4] (b_lo-major halves) and
            slot-order weight row [1, 2*npt*4].

            Per group slot i = pt*4 + q; storage row p = i%16 holds
            e = F*p + i//16 with F = npt/4.  cx/cy/wx/wy are (lo, hi)
            [1, *] rows; samples for (b_lo, axis) start at col
            xoff(b_lo, axis).
            """
            F = npt // 4
            ni = 2 * npt * 4
            idxe = pool.tile([1, ni], f32, tag=tagpfx + "idx")
            wrow = pool.tile([1, ni], f32, tag=tagpfx + "wrow")
            it, io = idxe[:].tensor, idxe[:].offset
            wt, wo = wrow[:].tensor, wrow[:].offset
            for b_lo in range(2):
                for q in range(4):
                    qy, qx = q // 2, q % 2
                    # e-grid: [pm = pt%4 (4), pd = pt//4 (F)]
                    ysrc = AP(cy[qy].tensor, cy[qy].offset + xoff(b_lo, 1),
                              [list(cy[qy].ap[0]), [1, 4], [4, F]])
                    xsrc = AP(cx[qx].tensor, cx[qx].offset + xoff(b_lo, 0),
                              [list(cx[qx].ap[0]), [1, 4], [4, F]])
                    idst = AP(it, io + b_lo * npt * 4 + F * q,
                              [[ni, 1], [4 * F, 4], [1, F]])
                    nc.vector.scalar_tensor_tensor(idst, ysrc, float(W),
                                                   xsrc, Alu.mult, Alu.add)
                    wysrc = AP(wy[qy].tensor, wy[qy].offset + xoff(b_lo, 1),
                               [list(wy[qy].ap[0]), [1, npt]])
                    wxsrc = AP(wx[qx].tensor, wx[qx].offset + xoff(b_lo, 0),
                               [list(wx[qx].ap[0]), [1, npt]])
                    wdst = AP(wt, wo + b_lo * npt * 4 + q,
                              [[ni, 1], [4, npt]])
                    nc.vector.tensor_mul(wdst, wysrc, wxsrc)
            return idxe, wrow

        def wrap_idx(pool, idx_row, ni, tagpfx):
            """e-order f32 idx row [1, ni] -> int16 idx tile [128, ni/32]."""
            nf = ni // 32
            wrapt = pool.tile([32, nf], f32, tag=tagpfx + "wrap")
            nc.sync.dma_start(wrapt[:], idx_row[:])
            rep_ps = psum1.tile([128, nf], f32, tag="replps")
            nc.tensor.matmul(rep_ps[:], replb_t[:], wrapt[:], start=True,
                             stop=True)
            idxt = pool.tile([128, nf], i16, tag=tagpfx + "idxi")
            nc.vector.tensor_copy(idxt[:], rep_ps[:])
            return idxt

        def wrap_w(pool, w_row, ni, tagpfx):
            """slot-order w row [1, ni] -> [2, ni/2] tile (b_lo rows)."""
            wpair = pool.tile([2, ni // 2], f32, tag=tagpfx + "wpair")
            nc.sync.dma_start(wpair[:], w_row[:])
            return wpair

        def repl_w(wpair, nf):
            wps = psum1.tile([128, nf], f32, tag="wps")
            nc.tensor.matmul(wps[:], onesb_t[:], wpair[:], start=True,
                             stop=True)
            return wps

        def combine(g, wps, npt, nslots):
            """g [128, nslots*4] (slot, cm) *= w[slot]; reduce over q ->
            [128, npt*4] cols (pt, cm)."""
            gv = g[:].rearrange("p (s c) -> p s c", c=4)
            wb = AP(wps[:].tensor, wps[:].offset,
                    [list(wps[:].ap[0]), [1, nslots], [0, 4]])
            nc.vector.tensor_mul(gv, gv, wb)
            red = gath.tile([128, npt * 4], f32, tag=f"red{nslots}")
            rin = AP(g[:].tensor, g[:].offset,
                     [list(g[:].ap[0]), [16, npt], [1, 4], [4, 4]])
            nc.vector.reduce_sum(red[:].rearrange("p (s c) -> p s c", c=4),
                                 rin, axis=mybir.AxisListType.X)
            return red

        # ---- pass-1 prep (coords only) ----
        base_v = base_t[:].rearrange("a (k x s n) -> a k x s n", k=NPAIR,
                                     x=2, n=4)
        idx1 = []
        w1pair = []
        for k in range(NPAIR):
            xy1 = rowp.tile([1, 4 * PT1], f32, tag="p1xy")
            nc.vector.memset(xy1[:], 0.0)
            for axis in range(2):
                for b_lo in range(2):
                    dst = xy1[:, axis * 2 * PT1 + b_lo * PT1:
                              axis * 2 * PT1 + b_lo * PT1 + J]
                    nc.vector.tensor_copy(
                        dst, base_v[:, k, axis, b_lo * J:(b_lo + 1) * J, 0])
            c0, c1, w0, w1_ = chain(rowp, xy1[:], 4 * PT1, "p1c")
            for wt_ in (w0, w1_):  # zero pad-point weights
                nc.vector.memset(
                    AP(wt_[:].tensor, wt_[:].offset + J,
                       [list(wt_[:].ap[0]), [PT1, 4], [1, PT1 - J]]), 0.0)
            idx_row, w_row = assemble(
                rowp, PT1, (c0[:], c1[:]), (c0[:], c1[:]),
                (w0[:], w1_[:]), (w0[:], w1_[:]),
                lambda b, axis: axis * 2 * PT1 + b * PT1, "p1a")
            idx1.append(wrap_idx(const, idx_row, 2 * N1, f"p1i{k}"))
            w1pair.append(wrap_w(const, w_row, 2 * N1, f"p1w{k}"))

        # ---- main loop over pairs ----
        for k in range(NPAIR):
            feat_t = featp.tile([128, 4 * FREE], f32, tag="feat")
            fpitch = feat_t[:].ap[0][0]
            for b_lo in range(2):
                dst = AP(feat_t[:].tensor,
                         feat_t[:].offset + b_lo * 64 * fpitch,
                         [[fpitch, 64], [1, 4 * FREE]])
                nc.sync.dma_start(dst, feats[2 * k + b_lo])

            if STAGE == "load":
                ot = outp.tile([PTN, C], f32, tag="ot")
                nc.vector.tensor_copy(ot[0:PTN, 0:C], feat_t[0:PTN, 0:C])
                nc.sync.dma_start(out_v[2 * k], ot[:])
                nc.sync.dma_start(out_v[2 * k + 1], ot[:])
                continue
            # pass 1: seed
            g1 = gath.tile([128, N1 * 4], f32, tag="g1")
            nc.gpsimd.ap_gather(g1[:], feat_t[:], idx1[k][:], channels=128,
                                num_elems=FREE, d=4, num_idxs=N1)
            seed = combine(g1, repl_w(w1pair[k], N1), PT1, N1)

            if STAGE == "g1":
                ot = outp.tile([PTN, C], f32, tag="ot")
                nc.vector.memset(ot[:], 0.0)
                nc.vector.tensor_copy(ot[0:PTN, 0:N1 * 4 // 8],
                                      seed[0:PTN, 0:N1 * 4 // 8])
                nc.sync.dma_start(out_v[2 * k], ot[:])
                nc.sync.dma_start(out_v[2 * k + 1], ot[:])
                continue
            # MLP layer 1: h = relu(w1 @ seed + b1)
            spitch = seed[:].ap[0][0]
            wpitch = w1q_t[:].ap[0][0]
            h_ps0 = psum1.tile([128, J], f32, tag="hps0")
            h_ps1 = psum1.tile([128, J], f32, tag="hps1")
            h_pss = [h_ps0, h_ps1]
            for b_lo in range(2):
                hsl = h_pss[b_lo][:]
                for cm in range(4):
                    rhs = AP(seed[:].tensor,
                             seed[:].offset + b_lo * 64 * spitch + cm,
                             [[spitch, 64], [4, J]])
                    lhsT = AP(w1q_t[:].tensor,
                              w1q_t[:].offset + b_lo * 64 * wpitch
                              + cm * 128,
                              [[wpitch, 64], [1, 128]])
                    nc.tensor.matmul(hsl, lhsT, rhs, start=(cm == 0),
                                     stop=(cm == 3))
            h_t = gath.tile([128, 2 * J], f32, tag="h")
            for b_lo in range(2):
                nc.scalar.activation(h_t[:, b_lo * J:(b_lo + 1) * J],
                                     h_pss[b_lo][:], Act.Relu,
                                     bias=b1_t[:, 0:1])

            if STAGE == "mlp":
                ot = outp.tile([PTN, C], f32, tag="ot")
                nc.vector.memset(ot[:], 0.0)
                nc.vector.tensor_copy(ot[0:PTN, 0:2 * J],
                                      h_t[0:PTN, 0:2 * J])
                nc.sync.dma_start(out_v[2 * k], ot[:])
                nc.sync.dma_start(out_v[2 * k + 1], ot[:])
                continue
            # layer 2 (transposed) + flatten to sample-major xy row
            offrow = rowp.tile([1, 4 * PTN], f32, tag="p2off")
            for b_lo in range(2):
                off_ps = psum1.tile([J, O2], f32, tag=f"offps{b_lo}")
                nc.tensor.matmul(off_ps[:], h_t[:, b_lo * J:(b_lo + 1) * J],
                                 w2_t[:], start=True, stop=True)
                off_t = gath.tile([J, O2], f32, tag=f"off{b_lo}")
                nc.vector.tensor_add(off_t[:], off_ps[:], b2_t[0:J, :])
                for axis in range(2):
                    src = AP(off_t[:].tensor, off_t[:].offset + axis,
                             [list(off_t[:].ap[0]), [2, NPTS]])
                    nc.sync.dma_start(
                        offrow[:, axis * 2 * PTN + b_lo * PTN:
                               axis * 2 * PTN + (b_lo + 1) * PTN], src)

            xy2 = rowp.tile([1, 4 * PTN], f32, tag="p2xy")
            nc.vector.tensor_add(
                xy2[:], offrow[:],
                base_t[:, k * 4 * PTN:(k + 1) * 4 * PTN])
            c0, c1, w0, w1_ = chain(rowp, xy2[:], 4 * PTN, "p2c")
            idx_row, w_row = assemble(
                rowp, PTN, (c0[:], c1[:]), (c0[:], c1[:]),
                (w0[:], w1_[:]), (w0[:], w1_[:]),
                lambda b, axis: axis * 2 * PTN + b * PTN, "p2a")
            idx2 = wrap_idx(wbp, idx_row, 2 * N2, "p2i")
            w2pair = wrap_w(wbp, w_row, 2 * N2, "p2w")

            if STAGE == "idx2":
                ot = outp.tile([PTN, C], f32, tag="ot")
                nc.vector.memset(ot[:], 0.0)
                nc.vector.tensor_copy(ot[0:PTN, 0:17],
                                      idx2[0:PTN, 0:17])
                nc.sync.dma_start(out_v[2 * k], ot[:])
                nc.sync.dma_start(out_v[2 * k + 1], ot[:])
                continue
            # pass 2: sample
            g2 = gath.tile([128, N2 * 4], f32, tag="g2")
            nc.gpsimd.ap_gather(g2[:], feat_t[:], idx2[:], channels=128,
                                num_elems=FREE, d=4, num_idxs=N2)
            samp = combine(g2, repl_w(w2pair, N2), PTN, N2)

            if STAGE == "g2":
                ot = outp.tile([PTN, C], f32, tag="ot")
                nc.vector.memset(ot[:], 0.0)
                nc.vector.tensor_copy(ot[0:PTN, 0:C], samp[0:PTN, 0:C])
                nc.sync.dma_start(out_v[2 * k], ot[:])
                nc.sync.dma_start(out_v[2 * k + 1], ot[:])
                continue
            # output: per (b_lo, cm) transpose [64, 68] -> [68, 64]
            gpitch = samp[:].ap[0][0]
            ipitch = ident2_t[:].ap[0][0]
            for b_lo in range(2):
                ot = outp.tile([PTN, C], f32, tag="ot")
                ov = ot[:].rearrange("q (cq cm) -> q cq cm", cm=4)
                for cm in range(4):
                    t_ps = psum1.tile([PTN, 64], f32, tag="tps")
                    lhsT = AP(samp[:].tensor,
                              samp[:].offset + b_lo * 64 * gpitch + cm,
                              [[gpitch, 64], [4, PTN]])
                    rhs = AP(ident2_t[:].tensor,
                             ident2_t[:].offset + b_lo * 64 * ipitch,
                             [[ipitch, 64], [1, 64]])
                    nc.tensor.matmul(t_ps[:], lhsT, rhs, is_transpose=True,
                                     start=True, stop=True)
                    nc.vector.tensor_copy(ov[:, :, cm], t_ps[:])
                nc.sync.dma_start(out_v[2 * k + b_lo], ot[:])

    nc.compile()
    return nc


def _host_prep(features, keypoint_coords, w1, b1, w2, b2):
    f32 = np.float32
    # channel-quad-last: [core, b, cq, (y, x, cm)]
    f = np.asarray(features, f32).reshape(NCORES, BPC, 64, 4, H, W)
    f = np.ascontiguousarray(f.transpose(0, 1, 2, 4, 5, 3))
    feats = f.reshape(NCORES, BPC, 64, 4 * H * W)

    pix = (np.asarray(keypoint_coords, f32) + 1.0) * 0.5 * (W - 1)
    bp = pix.reshape(NCORES, NPAIR, 2, J, 2)            # [core,k,b,pt,ax]
    bp = bp.transpose(0, 1, 4, 2, 3)                     # [core,k,ax,b,pt]
    bp = np.repeat(bp[..., None], NPTS, axis=-1)         # [...,n]
    bp = np.ascontiguousarray(
        bp.reshape(NCORES, 1, NPAIR * 2 * 2 * PTN), f32)

    w1T = np.asarray(w1, f32).T                          # [256, 128]
    w1q_half = np.empty((64, 512), f32)
    for cm in range(4):
        w1q_half[:, cm * 128:(cm + 1) * 128] = w1T[cm::4]
    w1q = np.ascontiguousarray(np.tile(w1q_half, (2, 1)))

    w2T = np.ascontiguousarray(np.asarray(w2, f32).T)
    b1c = np.ascontiguousarray(np.asarray(b1, f32)[:, None])
    b2c = np.ascontiguousarray(np.tile(np.asarray(b2, f32)[None, :],
                                       (128, 1)))
    P = np.arange(128)
    K = np.arange(32)
    replb = ((P[None, :] // 64 == K[:, None] // 16) &
             (P[None, :] % 16 == K[:, None] % 16)).astype(f32)
    onesb = (P[None, :] // 64 == np.arange(2)[:, None]).astype(f32)
    ident2 = np.ascontiguousarray(np.tile(np.eye(64, dtype=f32), (2, 1)))

    in_maps = []
    for i in range(NCORES):
        in_maps.append({
            "features": feats[i],
            "base_pix": bp[i],
            "w1q": w1q,
            "w2T": w2T,
            "b1": b1c,
            "b2": b2c,
            "replb": replb,
            "onesb": onesb,
            "ident2": ident2,
        })
    return in_maps


def kernel(features, keypoint_coords, w1, b1, w2, b2):
    global LAST_RESULTS
    from concourse.bass_utils import run_bass_kernel_spmd

    if "nc" not in _CACHE:
        _CACHE["nc"] = _build()
    nc = _CACHE["nc"]
    in_maps = _host_prep(features, keypoint_coords, w1, b1, w2, b2)
    res = run_bass_kernel_spmd(nc, in_maps, core_ids=list(range(NCORES)))
    LAST_RESULTS = res
    out = np.concatenate([res.results[i]["out"] for i in range(NCORES)],
                         axis=0)
    return out.astype(np.float32)


if __name__ == "__main__":
    nc = _build()
    print("build + compile OK")
